# revision 1
# baseline (speedup 1.0000x reference)
"""Trainium2 Bass kernel for nn_Diagnet (S=1024, B=64, I=512, H=2048, O=512).

    u = einsum('sbi,hi->sbh', X, W_ih)
    h_{t} = |u_t + hh * h_{t-1}|   (scan over S, only final h needed)
    Y = h_final @ W_ho.T + b_ho

Strategy (8 NeuronCores, data-parallel over batch, 8 batch rows per core):

* H lanes are permuted so hh is sorted descending and split into 16
  chunks of 128.  The recurrence is a contraction with per-lane factor
  a=hh<1, so a chunk whose largest a satisfies a^K < 1e-10 only needs
  the last K steps: the input->hidden GEMM and the scan skip everything
  earlier (this is exact to ~1e-10 relative, far below fp32 noise).
* Within each 64-step block the state is kept pre-scaled as
  m_tau = a^(63-tau) * h.  Then the step is a multiply-free
  m = |m + a^(63-tau) u_t|, applied by a custom fused DVE op
  (out = |in0 + in1|), one instruction per step over all active chunks.
  Entering a block multiplies the state once by a^64.  Scales a^(63-tau)
  fold into the PSUM->SBUF move of the GEMM output (one tensor_tensor
  multiply).  Underflow of a^64 for small-a lanes reproduces the
  truncation automatically, and no overflow is possible (scales <= 1).
* GEMM: X is pre-tiled host-side into [block, i-chunk, 128, (b,t)]
  (contraction dim on partitions), multiplied against host-transposed
  W_ih^T in fp32.  PSUM layout [h, (b,t)] hands each scan step a
  contiguous slice after a fused scale+move to SBUF.
* Final projection: h_final tiles (already [h,b] on chip) are the
  stationary operand against host-transposed W_ho^T; bias added on DVE.
"""

import math
import os

from contextlib import ExitStack

import numpy as np

S, B, I, H, O = 1024, 64, 512, 2048, 512
NCORES = 8
BC = B // NCORES  # 8 batch rows per core
TB = 64  # time block == scan window
NBLK = S // TB  # 16
NCH = H // 128  # 16 h-chunks
LN_TRUNC = 23.03  # a^K <= e^-23 ~ 1e-10 -> truncate

_CACHE = {}


def _register_abs_add():
    import concourse.dve_ops as dve_ops
    from concourse.dve_spec import Spec, Src0, Src1, Zero, maxx, lower
    from concourse.dve_uop import DveOpSpec

    for op in dve_ops.OPS:
        if op.name == "ABS_ADD_ANT":
            return op
    x = Src0 + Src1
    spec = Spec(
        body=maxx(x, Zero - x),
        reference=lambda in0, in1, s0, s1, imm2: np.abs(
            in0.astype(np.float32) + in1.astype(np.float32)
        ),
    )
    row = max(dve_ops._SUB_OPCODE_FOR_NAME.values()) + 1
    assert row < 0x20
    shas = {}
    for ver in ("v3", "v4"):
        s = DveOpSpec(name="ABS_ADD_ANT", opcode=row, uops=lower(spec, ver=ver), rd1_en=True)
        shas[ver] = s.sha(ver)
    op = dve_ops.DveOp("ABS_ADD_ANT", spec, subdim=False, uops_sha=shas)
    dve_ops._SUB_OPCODE_FOR_NAME["ABS_ADD_ANT"] = row
    dve_ops.OPS.append(op)
    dve_ops.CUSTOM_DVE_SPECS["ABS_ADD_ANT"] = spec
    return op


def _make_plan(hh):
    a = np.maximum(np.abs(hh.astype(np.float64)), 1e-30)
    # jax uniform is [0,1); abs is a no-op safeguard.
    perm = np.argsort(-a, kind="stable")
    a_s = a[perm]
    first_block = []
    for g in range(NCH):
        amax = a_s[g * 128]
        if amax >= math.exp(-LN_TRUNC / S):
            kg = S
        else:
            kg = min(S, int(math.ceil(LN_TRUNC / math.log(1.0 / amax))))
        kg = min(S, ((kg + TB - 1) // TB) * TB)
        first_block.append(NBLK - kg // TB)
    # chunks sorted by a desc -> first_block nondecreasing -> active set is
    # always a chunk prefix.
    assert all(
        first_block[g] <= first_block[g + 1] for g in range(NCH - 1)
    ), first_block
    ag = a_s.reshape(NCH, 128).T  # [128, NCH] lane a per chunk
    tau = np.arange(TB)
    sc = ag[:, :, None] ** (TB - 1 - tau)[None, None, :]  # [128, NCH, TB]
    a64 = np.repeat(ag**TB, BC, axis=1)  # [128, NCH*BC]
    return {
        "perm": perm,
        "first_block": tuple(first_block),
        "SC": sc.reshape(128, NCH * TB).astype(np.float32),
        "A64": a64.astype(np.float32),
    }


def _build(first_block, use_f32r):
    import concourse.mybir as mybir
    import concourse.tile as tile
    from concourse import bacc
    from concourse.bass import ds

    ABS_ADD = _register_abs_add()
    f32 = mybir.dt.float32
    gemm_dt = mybir.dt.float32r if use_f32r else f32

    nc = bacc.Bacc("TRN2", target_bir_lowering=False, debug=False, num_devices=NCORES)
    X = nc.dram_tensor("X", [NBLK, I // 128, 128, TB * BC], gemm_dt, kind="ExternalInput").ap()
    WIHT = nc.dram_tensor("WIHT", [I, H], gemm_dt, kind="ExternalInput").ap()
    WHOT = nc.dram_tensor("WHOT", [H, O], f32, kind="ExternalInput").ap()
    BIAS = nc.dram_tensor("BIAS", [BC, O], f32, kind="ExternalInput").ap()
    SC = nc.dram_tensor("SC", [128, NCH * TB], f32, kind="ExternalInput").ap()
    A64 = nc.dram_tensor("A64", [128, NCH * BC], f32, kind="ExternalInput").ap()
    Y = nc.dram_tensor("Y", [BC, O], f32, kind="ExternalOutput").ap()

    NI = I // 128  # 4 i-chunks

    with tile.TileContext(nc) as tc:
        with ExitStack() as ctx:
            consts = ctx.enter_context(tc.tile_pool(name="consts", bufs=1))
            xtpool = ctx.enter_context(tc.tile_pool(name="xt", bufs=3))
            upool = ctx.enter_context(tc.tile_pool(name="ubuf", bufs=1))
            ypool = ctx.enter_context(tc.tile_pool(name="yout", bufs=1))
            gpool = ctx.enter_context(tc.tile_pool(name="gpsum", bufs=int(os.environ.get("DIAG_GP", "4")), space="PSUM"))
            fpool = ctx.enter_context(tc.tile_pool(name="fpsum", bufs=1, space="PSUM"))

            # constants
            wiht = [consts.tile([128, H], gemm_dt, tag=f"wiht{ic}", name=f"wiht{ic}") for ic in range(NI)]
            for ic in range(NI):
                nc.sync.dma_start(wiht[ic][:], WIHT[ds(ic * 128, 128), :])
            sc_t = consts.tile([128, NCH * TB], f32, tag="sc", name="sc_t")
            nc.sync.dma_start(sc_t[:], SC)
            a64_t = consts.tile([128, NCH * BC], f32, tag="a64", name="a64_t")
            nc.sync.dma_start(a64_t[:], A64)
            m_t = consts.tile([128, NCH * BC], f32, tag="state", name="m_t")
            nc.vector.memset(m_t[:], 0.0)

            acts = [sum(1 for fb in first_block if fb <= kb) for kb in range(NBLK)]
            assert all(a >= 1 for a in acts)
            u_tiles = [None] * NBLK

            def produce(kb):
                act = acts[kb]
                # --- load pre-transposed X tiles [i, (b,t)] ---
                xt = []
                for ic in range(NI):
                    xt_ic = xtpool.tile([128, TB * BC], gemm_dt, tag=f"xt{ic}", name=f"xt_{kb}_{ic}")
                    nc.sync.dma_start(xt_ic[:], X[kb, ic])
                    xt.append(xt_ic)
                # u buffer for this block: [128, (tau, active-chunk, b)]
                u_t = upool.tile([128, TB * act * BC], f32, tag=f"u{kb}", name=f"u_{kb}")
                u_tiles[kb] = u_t
                for g in range(act):
                    ps = gpool.tile([128, TB * BC], f32, tag="gp", name=f"gp_{kb}_{g}")
                    for ic in range(NI):
                        nc.tensor.matmul(
                            ps[:],
                            wiht[ic][:, ds(g * 128, 128)],
                            xt[ic][:],
                            start=(ic == 0),
                            stop=(ic == NI - 1),
                        )
                    # scaled move psum->sbuf:
                    # u_t[p, tau*act*BC + g*BC + b] = ps[p, b*TB+tau]*SC[p,g*TB+tau]
                    dst = u_t[:].rearrange("p (t c) -> p t c", t=TB)[
                        :, :, ds(g * BC, BC)
                    ]
                    srcp = ps[:].rearrange("p (b t) -> p t b", b=BC)
                    scl = sc_t[:, ds(g * TB, TB)].broadcast_to([128, TB, BC])
                    nc.vector.tensor_tensor(dst, srcp, scl, mybir.AluOpType.mult)

            def scan(kb):
                act = acts[kb]
                na = act * BC
                u_t = u_tiles[kb]
                nc.gpsimd.tensor_tensor(
                    m_t[:, 0:na], m_t[:, 0:na], a64_t[:, 0:na], mybir.AluOpType.mult
                )
                for tau in range(TB):
                    nc.vector._custom_dve(
                        ABS_ADD,
                        out=m_t[:, 0:na],
                        in0=m_t[:, 0:na],
                        in1=u_t[:, ds(tau * act * BC, na)],
                    )

            LAG = int(os.environ.get("DIAG_LAG", "2"))
            if os.environ.get("DIAG_ORDER", "seq") == "front":
                # front-load the heaviest (latest) blocks' GEMMs to keep the
                # PE dense/warm while the serial scan chain progresses.
                heavy = [NBLK - 1, NBLK - 2]
                order = heavy + [kb for kb in range(NBLK) if kb not in heavy]
            else:
                order = list(range(NBLK))
            scanned = 0
            produced = set()

            def scan_ready_upto(limit):
                nonlocal scanned
                while scanned < limit and scanned in produced:
                    scan(scanned)
                    scanned += 1

            for i, kb in enumerate(order):
                produce(kb)
                produced.add(kb)
                scan_ready_upto(i + 1 - LAG)
            scan_ready_upto(NBLK)
            assert scanned == NBLK

            # --- final projection: Y = h^T @ WHOT + bias ---
            whot = [consts.tile([128, O], f32, tag=f"whot{g}", name=f"whot{g}") for g in range(NCH)]
            for g in range(NCH):
                nc.sync.dma_start(whot[g][:], WHOT[ds(g * 128, 128), :])
            bias_t = ypool.tile([BC, O], f32, tag="bias", name="bias_t")
            nc.sync.dma_start(bias_t[:], BIAS)
            psy = fpool.tile([BC, O], f32, tag="fy", name="psy")
            for g in range(NCH):
                nc.tensor.matmul(
                    psy[:],
                    m_t[:, ds(g * BC, BC)],
                    whot[g][:],
                    start=(g == 0),
                    stop=(g == NCH - 1),
                )
            y_t = ypool.tile([BC, O], f32, tag="y", name="y_t")
            nc.vector.tensor_tensor(y_t[:], psy[:], bias_t[:], mybir.AluOpType.add)
            nc.sync.dma_start(Y, y_t[:])
    nc.compile()
    return nc


def _get_program(first_block, use_f32r):
    key = (first_block, use_f32r, os.environ.get("DIAG_LAG"), os.environ.get("DIAG_GP"), os.environ.get("DIAG_ORDER"))
    if key not in _CACHE:
        _CACHE[key] = _build(first_block, use_f32r)
    return _CACHE[key]


def _round_f32r(x):
    """Round fp32 array to fp32r (s8e11) representable values."""
    u = np.ascontiguousarray(x).view(np.uint32)
    r = ((u.astype(np.uint64) + 0x800) & 0xFFFFF000).astype(np.uint32)
    return r.view(np.float32).reshape(x.shape)


def _ensure_ntff_hook():
    """Provide antenv.axon_hooks (absent in this image) so trace=True works."""
    import sys
    import types

    if "antenv.axon_hooks" in sys.modules:
        return True
    try:
        import antenv

        mod = types.ModuleType("antenv.axon_hooks")
        mod._hook = None

        def set_axon_ntff_profile_hook(h):
            mod._hook = h

        def get_axon_ntff_profile_hook():
            return mod._hook

        mod.set_axon_ntff_profile_hook = set_axon_ntff_profile_hook
        mod.get_axon_ntff_profile_hook = get_axon_ntff_profile_hook
        sys.modules["antenv.axon_hooks"] = mod
        antenv.axon_hooks = mod

        from trn_agent_boot.trn_boot import _ntff_profile_via_ctypes

        hook = _ntff_profile_via_ctypes("/opt/axon/libaxon_pjrt.so")
        mod.set_axon_ntff_profile_hook(hook)
        return hook is not None
    except Exception:
        return False


def kernel(X, W_ih, hh, W_ho, b_ho):
    from concourse import bass_utils

    X = np.asarray(X, dtype=np.float32)
    W_ih = np.asarray(W_ih, dtype=np.float32)
    hh = np.asarray(hh, dtype=np.float32)
    W_ho = np.asarray(W_ho, dtype=np.float32)
    b_ho = np.asarray(b_ho, dtype=np.float32)

    use_f32r = bool(int(os.environ.get("DIAG_F32R", "0")))
    plan = _make_plan(hh)
    perm = plan["perm"]
    nc = _get_program(plan["first_block"], use_f32r)

    wiht = np.ascontiguousarray(W_ih[perm].T)  # [I, H]
    if use_f32r:
        wiht = _round_f32r(wiht)
    whot = np.ascontiguousarray(W_ho[:, perm].T)  # [H, O]
    bias = np.tile(b_ho[None, :], (BC, 1)).astype(np.float32)

    common = {
        "WIHT": wiht,
        "WHOT": whot,
        "BIAS": bias,
        "SC": plan["SC"],
        "A64": plan["A64"],
    }
    in_maps = []
    for m in range(NCORES):
        im = dict(common)
        xm = X[:, m * BC : (m + 1) * BC, :]  # [S, BC, I]
        # device tile layout [NBLK, NI, 128(i), (b, tau)]
        xt = xm.transpose(2, 1, 0).reshape(I // 128, 128, BC, NBLK, TB)
        xt = np.ascontiguousarray(xt.transpose(3, 0, 1, 2, 4)).reshape(
            NBLK, I // 128, 128, TB * BC
        )
        if use_f32r:
            xt = _round_f32r(xt)
        im["X"] = xt
        in_maps.append(im)

    trace = bool(int(os.environ.get("DIAG_TRACE", "0")))
    if trace:
        trace = _ensure_ntff_hook()
    res = None
    for attempt in range(3):
        try:
            res = bass_utils.run_bass_kernel_spmd(
                nc,
                in_maps,
                core_ids=list(range(NCORES)),
                trace=trace,
                tmpdir=os.environ.get("DIAG_TRACE_DIR") or None,
            )
            break
        except Exception:
            if attempt == 2:
                raise
            trace = False  # retry without profiling
    if res.exec_time_ns is not None:
        kernel.last_exec_time_ns = res.exec_time_ns
        kernel.last_mean_exec_time_ns = res.mean_exec_time_ns
    Yfull = np.concatenate([r["Y"] for r in res.results], axis=0)
    return Yfull


kernel.last_exec_time_ns = None
kernel.last_mean_exec_time_ns = None



# revision 3
# speedup vs baseline: 2.5980x; 2.5980x over previous
"""Trainium2 Bass kernel for nn_Diagnet (S=1024, B=64, I=512, H=2048, O=512).

    u = einsum('sbi,hi->sbh', X, W_ih)
    h_t = |u_t + hh * h_{t-1}|   (scan over S, only final h needed)
    Y = h_final @ W_ho.T + b_ho

Strategy (8 NeuronCores, data-parallel over batch, BC=8 rows per core):

* H lanes are permuted so hh is sorted descending and split into 16
  chunks of 128.  A chunk whose largest decay a satisfies a^K < tol
  only needs the last K steps (exact to ~tol relative), so each chunk
  gets a window K_g (multiple of 64), and the GEMM + scan skip
  everything earlier.
* The recurrence is computed by a custom DVE instruction that folds
  the WHOLE window in one go: out[tau] = |out[tau-1] - u[tau]*scn[tau]|
  via scan(ABSOLUTE_DIFF, Src0*Src1).  The running state lives in the
  engine (no SBUF round-trip per step), so the serial chain that
  dominated the naive per-step implementation (~200ns x 1024 steps)
  collapses to one ~K-cycle streaming instruction per (chunk, batch).
  scn[tau] = -a_lane^(K-1-tau) folds the per-step decay multiply into
  a prescale (a>=0 lets a*|x| = |a x|), and the minus sign turns
  ABSOLUTE_DIFF into abs-add.  h_final = last scan element (scale 1).
* GEMM runs in bf16 (1 cycle/row on the PE vs 4 for fp32; X DMA
  halves).  X is host-tiled to [block, i-chunk, 128i, (b,tau)] and
  kept resident in SBUF; the GEMM iterates chunk-major (longest
  window first) so each chunk's scan overlaps later chunks' GEMMs,
  with i-chunk-outer PSUM accumulation runs to amortize LDWEIGHTS.
* The Activation engine drains PSUM -> SBUF with a pure layout copy
  (to b-major contiguous windows); GPSIMD extracts h_final columns
  (cast to bf16); the final projection is 16 accumulating bf16
  matmuls + bias add at the end.
"""

import math
import os

from contextlib import ExitStack

import numpy as np

S, B, I, H, O = 1024, 64, 512, 2048, 512
NCORES = 8
BC = B // NCORES  # 8 batch rows per core
TB = 64  # block granularity for truncation windows
NBLK = S // TB  # 16
NCH = H // 128  # 16 h-chunks
NI = I // 128  # 4 i-chunks
USMALL_W = 256  # max window (cols) for chunks g>=1; K_1 <= 256 needs LN <= ~16

_CACHE = {}


def _register_scan_op():
    import concourse.dve_ops as dve_ops
    from concourse.dve_spec import Spec, Src0, Src1, Zero, scan, lower, AluOp
    from concourse.dve_uop import DveOpSpec

    for op in dve_ops.OPS:
        if op.name == "ABSDIFF_SCALE_SCAN_ANT":
            return op

    def _ref(in0, in1, s0, s1, imm2):
        x = in0.astype(np.float32) * in1.astype(np.float32)
        out = np.empty_like(x)
        m = np.zeros(x.shape[0], np.float32)
        for t in range(x.shape[1]):
            m = np.abs(m - x[:, t])
            out[:, t] = m
        return out

    spec = Spec(
        body=scan(AluOp.ABSOLUTE_DIFF, Src0 * Src1, init=Zero),
        reference=_ref,
    )
    row = max(dve_ops._SUB_OPCODE_FOR_NAME.values()) + 1
    assert row < 0x20
    shas = {}
    for ver in ("v3", "v4"):
        s = DveOpSpec(
            name="ABSDIFF_SCALE_SCAN_ANT", opcode=row, uops=lower(spec, ver=ver), rd1_en=True
        )
        shas[ver] = s.sha(ver)
    op = dve_ops.DveOp("ABSDIFF_SCALE_SCAN_ANT", spec, subdim=False, uops_sha=shas)
    dve_ops._SUB_OPCODE_FOR_NAME["ABSDIFF_SCALE_SCAN_ANT"] = row
    dve_ops.OPS.append(op)
    dve_ops.CUSTOM_DVE_SPECS["ABSDIFF_SCALE_SCAN_ANT"] = spec
    return op


def _make_plan(hh):
    ln = float(os.environ.get("DIAG_LN", "9.2"))  # a^K <= e^-ln truncation tol
    a = np.maximum(np.abs(hh.astype(np.float64)), 1e-30)
    perm = np.argsort(-a, kind="stable")
    ag = a[perm].reshape(NCH, 128)  # [chunk, lane], descending
    windows = []
    for g in range(NCH):
        amax = ag[g, 0]
        if S * math.log(amax) >= -ln:
            kg = S
        else:
            kg = int(math.ceil(ln / math.log(1.0 / amax)))
        kg = min(S, max(TB, ((kg + TB - 1) // TB) * TB))
        windows.append(kg)
    assert all(windows[g] >= windows[g + 1] for g in range(NCH - 1)), windows
    assert all(k <= USMALL_W for k in windows[1:]), (windows, "raise USMALL_W")
    offs = np.concatenate([[0], np.cumsum(windows)]).astype(int)
    scn = np.zeros((128, offs[-1]), dtype=np.float64)
    for g in range(NCH):
        kg = windows[g]
        tau = np.arange(kg)
        scn[:, offs[g] : offs[g] + kg] = -(ag[g][:, None] ** (kg - 1 - tau)[None, :])
    return {
        "perm": perm,
        "windows": tuple(windows),
        "offs": offs,
        "SCN": scn.astype(np.float32),
    }


def _build(windows, offs_total):
    import concourse.mybir as mybir
    import concourse.tile as tile
    from concourse import bacc
    from concourse.bass import ds

    SCAN_OP = _register_scan_op()
    f32 = mybir.dt.float32
    bf16 = mybir.dt.bfloat16
    R = int(os.environ.get("DIAG_R", "6"))

    nc = bacc.Bacc("TRN2", target_bir_lowering=False, debug=False, num_devices=NCORES)
    X = nc.dram_tensor("X", [NBLK, NI, 128, TB * BC], bf16, kind="ExternalInput").ap()
    WIHT = nc.dram_tensor("WIHT", [I, H], bf16, kind="ExternalInput").ap()
    WHOT = nc.dram_tensor("WHOT", [H, O], bf16, kind="ExternalInput").ap()
    SCN = nc.dram_tensor("SCN", [128, offs_total], f32, kind="ExternalInput").ap()
    BIAS = nc.dram_tensor("BIAS", [BC, O], f32, kind="ExternalInput").ap()
    Y = nc.dram_tensor("Y", [BC, O], f32, kind="ExternalOutput").ap()

    offs = np.concatenate([[0], np.cumsum(windows)]).astype(int)

    with tile.TileContext(nc) as tc:
        with ExitStack() as ctx:
            consts = ctx.enter_context(tc.tile_pool(name="consts", bufs=1))
            xpool = ctx.enter_context(tc.tile_pool(name="xt", bufs=1))
            ubig = ctx.enter_context(tc.tile_pool(name="ubig", bufs=1))
            usmall = ctx.enter_context(tc.tile_pool(name="usmall", bufs=4))
            ypool = ctx.enter_context(tc.tile_pool(name="yout", bufs=1))
            gpool = ctx.enter_context(tc.tile_pool(name="gpsum", bufs=7, space="PSUM"))
            fpool = ctx.enter_context(tc.tile_pool(name="fpsum", bufs=1, space="PSUM"))

            # --- constants / inputs ---
            wiht = [
                consts.tile([128, H], bf16, tag=f"wiht{ic}", name=f"wiht{ic}")
                for ic in range(NI)
            ]
            for ic in range(NI):
                nc.sync.dma_start(wiht[ic][:], WIHT[ds(ic * 128, 128), :])
            scn_t = consts.tile([128, offs_total], f32, tag="scn", name="scn_t")
            nc.sync.dma_start(scn_t[:], SCN)
            xt = [
                [
                    xpool.tile([128, TB * BC], bf16, tag=f"x{kb}_{ic}", name=f"x_{kb}_{ic}")
                    for ic in range(NI)
                ]
                for kb in range(NBLK)
            ]
            for kb in range(NBLK):
                for ic in range(NI):
                    nc.sync.dma_start(xt[kb][ic][:], X[kb, ic])
            bias_t = ypool.tile([BC, O], f32, tag="bias", name="bias_t")
            nc.sync.dma_start(bias_t[:], BIAS)
            whot = [
                consts.tile([128, O], bf16, tag=f"whot{g}", name=f"whot{g}")
                for g in range(NCH)
            ]
            for g in range(NCH):
                nc.sync.dma_start(whot[g][:], WHOT[ds(g * 128, 128), :])

            h_all = consts.tile([128, NCH * BC], bf16, tag="hall", name="h_all")

            # --- chunk-major pipeline: GEMM (PE) -> copy (ACT) -> scan (DVE) ---
            for g in range(NCH):
                kg = windows[g]
                nbg = kg // TB
                fb = NBLK - nbg
                if g == 0:
                    u_t = ubig.tile([128, BC * kg], f32, tag="u0", name="u_g0")
                else:
                    u_t = usmall.tile(
                        [128, BC * USMALL_W], f32, tag="us", name=f"u_g{g}"
                    )
                uwin = u_t[:, ds(0, BC * kg)]
                for rs in range(fb, NBLK, R):
                    run = list(range(rs, min(rs + R, NBLK)))
                    ps = {
                        kb: gpool.tile([128, TB * BC], f32, tag="gp", name=f"gp_{g}_{kb}")
                        for kb in run
                    }
                    for ic in range(NI):
                        for kb in run:
                            nc.tensor.matmul(
                                ps[kb][:],
                                wiht[ic][:, ds(g * 128, 128)],
                                xt[kb][ic][:],
                                start=(ic == 0),
                                stop=(ic == NI - 1),
                            )
                    for kb in run:
                        j = kb - fb
                        dst = uwin.rearrange("p (b t) -> p t b", b=BC)[
                            :, ds(j * TB, TB), :
                        ]
                        src = ps[kb][:].rearrange("p (b t) -> p t b", b=BC)
                        nc.scalar.copy(dst, src)
                # scans: one instruction per batch row, whole window
                scn_g = scn_t[:, ds(int(offs[g]), kg)]
                for b in range(BC):
                    ap = u_t[:, ds(b * kg, kg)]
                    nc.vector._custom_dve(SCAN_OP, out=ap, in0=ap, in1=scn_g)
                # h_final = last scan element per (lane, b) -> bf16
                hsrc = uwin.rearrange("p (b t) -> p b t", b=BC)[:, :, kg - 1]
                nc.gpsimd.tensor_copy(h_all[:, ds(g * BC, BC)], hsrc)

            # --- final projection: Y = h^T @ WHOT + bias ---
            psy = fpool.tile([BC, O], f32, tag="fy", name="psy")
            for g in range(NCH):
                nc.tensor.matmul(
                    psy[:],
                    h_all[:, ds(g * BC, BC)],
                    whot[g][:],
                    start=(g == 0),
                    stop=(g == NCH - 1),
                )
            y_t = ypool.tile([BC, O], f32, tag="y", name="y_t")
            nc.vector.tensor_tensor(y_t[:], psy[:], bias_t[:], mybir.AluOpType.add)
            nc.sync.dma_start(Y, y_t[:])
    nc.compile()
    return nc


def _get_program(windows, offs_total):
    key = (
        windows,
        os.environ.get("DIAG_R"),
        os.environ.get("DIAG_LN"),
    )
    if key not in _CACHE:
        _CACHE[key] = _build(windows, offs_total)
    return _CACHE[key]


def _ensure_ntff_hook():
    """Provide antenv.axon_hooks (absent in this image) so trace=True works."""
    import sys
    import types

    if "antenv.axon_hooks" in sys.modules:
        return True
    try:
        import antenv

        mod = types.ModuleType("antenv.axon_hooks")
        mod._hook = None

        def set_axon_ntff_profile_hook(h):
            mod._hook = h

        def get_axon_ntff_profile_hook():
            return mod._hook

        mod.set_axon_ntff_profile_hook = set_axon_ntff_profile_hook
        mod.get_axon_ntff_profile_hook = get_axon_ntff_profile_hook
        sys.modules["antenv.axon_hooks"] = mod
        antenv.axon_hooks = mod

        from trn_agent_boot.trn_boot import _ntff_profile_via_ctypes

        hook = _ntff_profile_via_ctypes("/opt/axon/libaxon_pjrt.so")
        mod.set_axon_ntff_profile_hook(hook)
        return hook is not None
    except Exception:
        return False


def kernel(X, W_ih, hh, W_ho, b_ho):
    import ml_dtypes

    from concourse import bass_utils

    X = np.asarray(X, dtype=np.float32)
    W_ih = np.asarray(W_ih, dtype=np.float32)
    hh = np.asarray(hh, dtype=np.float32)
    W_ho = np.asarray(W_ho, dtype=np.float32)
    b_ho = np.asarray(b_ho, dtype=np.float32)

    plan = _make_plan(hh)
    perm = plan["perm"]
    nc = _get_program(plan["windows"], int(plan["offs"][-1]))

    bf = ml_dtypes.bfloat16
    wiht = np.ascontiguousarray(W_ih[perm].T).astype(bf)  # [I, H]
    whot = np.ascontiguousarray(W_ho[:, perm].T).astype(bf)  # [H, O]
    bias = np.tile(b_ho[None, :], (BC, 1)).astype(np.float32)

    common = {
        "WIHT": wiht,
        "WHOT": whot,
        "BIAS": bias,
        "SCN": plan["SCN"],
    }
    in_maps = []
    for m in range(NCORES):
        im = dict(common)
        xm = X[:, m * BC : (m + 1) * BC, :]  # [S, BC, I]
        # device tile layout [NBLK, NI, 128(i), (b, tau)]
        xt = xm.transpose(2, 1, 0).reshape(NI, 128, BC, NBLK, TB)
        xt = np.ascontiguousarray(xt.transpose(3, 0, 1, 2, 4)).reshape(
            NBLK, NI, 128, TB * BC
        )
        im["X"] = xt.astype(bf)
        in_maps.append(im)

    trace = bool(int(os.environ.get("DIAG_TRACE", "0")))
    if trace:
        trace = _ensure_ntff_hook()
    res = None
    for attempt in range(3):
        try:
            res = bass_utils.run_bass_kernel_spmd(
                nc,
                in_maps,
                core_ids=list(range(NCORES)),
                trace=trace,
                tmpdir=os.environ.get("DIAG_TRACE_DIR") or None,
            )
            break
        except Exception:
            if attempt == 2:
                raise
            trace = False  # retry without profiling
    if res.exec_time_ns is not None:
        kernel.last_exec_time_ns = res.exec_time_ns
        kernel.last_mean_exec_time_ns = res.mean_exec_time_ns
    Yfull = np.concatenate([r["Y"] for r in res.results], axis=0)
    return Yfull


kernel.last_exec_time_ns = None
kernel.last_mean_exec_time_ns = None


# revision 12
# speedup vs baseline: 3.2113x; 1.2361x over previous
"""Trainium2 Bass kernel for nn_Diagnet (S=1024, B=64, I=512, H=2048, O=512).

    u = einsum('sbi,hi->sbh', X, W_ih)
    h_t = |u_t + hh * h_{t-1}|   (scan over S, only final h needed)
    Y = h_final @ W_ho.T + b_ho

Strategy (8 NeuronCores, data-parallel over batch, BC=8 rows per core):

* H lanes are permuted so hh is sorted descending and split into 16
  chunks of 128.  A chunk whose largest decay a satisfies a^K < tol
  only needs the last K steps (exact to ~tol relative), so each chunk
  gets a window K_g (multiple of 64), and the GEMM + scan skip
  everything earlier.
* The recurrence is computed by a custom DVE instruction that folds
  the WHOLE window in one go: out[tau] = |out[tau-1] - u[tau]*scn[tau]|
  via scan(ABSOLUTE_DIFF, Src0*Src1).  The running state lives in the
  engine (no SBUF round-trip per step), so the serial chain that
  dominated the naive per-step implementation (~200ns x 1024 steps)
  collapses to one ~K-cycle streaming instruction per (chunk, batch).
  scn[tau] = -a_lane^(K-1-tau) folds the per-step decay multiply into
  a prescale (a>=0 lets a*|x| = |a x|), and the minus sign turns
  ABSOLUTE_DIFF into abs-add.  h_final = last scan element (scale 1).
* GEMM runs in bf16 (1 cycle/row on the PE vs 4 for fp32; X DMA
  halves).  X is host-tiled to [block, i-chunk, 128i, (b,tau)] and
  kept resident in SBUF; the GEMM iterates chunk-major (longest
  window first) so each chunk's scan overlaps later chunks' GEMMs,
  with i-chunk-outer PSUM accumulation runs to amortize LDWEIGHTS.
* The Activation engine drains PSUM -> SBUF with a pure layout copy
  (to b-major contiguous windows); GPSIMD extracts h_final columns
  (cast to bf16); the final projection is 16 accumulating bf16
  matmuls + bias add at the end.
"""

import math
import os

from contextlib import ExitStack

import numpy as np

S, B, I, H, O = 1024, 64, 512, 2048, 512
NCORES = 8
BC = B // NCORES  # 8 batch rows per core
TB = 64  # block granularity for truncation windows
NBLK = S // TB  # 16
NCH = H // 128  # 16 h-chunks
NI = I // 128  # 4 i-chunks
USMALL_W = 256  # max window (cols) for chunks g>=1; K_1 <= 256 needs LN <= ~16

_CACHE = {}


def _register_scan_op():
    import concourse.dve_ops as dve_ops
    from concourse.dve_spec import Spec, Src0, Src1, Zero, scan, lower, AluOp
    from concourse.dve_uop import DveOpSpec

    for op in dve_ops.OPS:
        if op.name == "ABSDIFF_SCALE_SCAN_ANT":
            return op

    def _ref(in0, in1, s0, s1, imm2):
        x = in0.astype(np.float32) * in1.astype(np.float32)
        out = np.empty_like(x)
        m = np.zeros(x.shape[0], np.float32)
        for t in range(x.shape[1]):
            m = np.abs(m - x[:, t])
            out[:, t] = m
        return out

    spec = Spec(
        body=scan(AluOp.ABSOLUTE_DIFF, Src0 * Src1, init=Zero),
        reference=_ref,
    )
    row = max(dve_ops._SUB_OPCODE_FOR_NAME.values()) + 1
    assert row < 0x20
    shas = {}
    for ver in ("v3", "v4"):
        s = DveOpSpec(
            name="ABSDIFF_SCALE_SCAN_ANT", opcode=row, uops=lower(spec, ver=ver), rd1_en=True
        )
        shas[ver] = s.sha(ver)
    op = dve_ops.DveOp("ABSDIFF_SCALE_SCAN_ANT", spec, subdim=False, uops_sha=shas)
    dve_ops._SUB_OPCODE_FOR_NAME["ABSDIFF_SCALE_SCAN_ANT"] = row
    dve_ops.OPS.append(op)
    dve_ops.CUSTOM_DVE_SPECS["ABSDIFF_SCALE_SCAN_ANT"] = spec
    return op


def _make_plan(hh):
    ln = float(os.environ.get("DIAG_LN", "9.2"))  # a^K <= e^-ln truncation tol
    a = np.maximum(np.abs(hh.astype(np.float64)), 1e-30)
    perm = np.argsort(-a, kind="stable")
    ag = a[perm].reshape(NCH, 128)  # [chunk, lane], descending
    windows = []
    for g in range(NCH):
        amax = ag[g, 0]
        if S * math.log(amax) >= -ln:
            kg = S
        else:
            kg = int(math.ceil(ln / math.log(1.0 / amax)))
        kg = min(S, max(TB, ((kg + TB - 1) // TB) * TB))
        windows.append(kg)
    assert all(windows[g] >= windows[g + 1] for g in range(NCH - 1)), windows
    assert all(k <= USMALL_W for k in windows[1:]), (windows, "raise USMALL_W")
    offs = np.concatenate([[0], np.cumsum(windows)]).astype(int)
    scn = np.zeros((128, offs[-1]), dtype=np.float64)
    for g in range(NCH):
        kg = windows[g]
        tau = np.arange(kg)
        scn[:, offs[g] : offs[g] + kg] = -(ag[g][:, None] ** (kg - 1 - tau)[None, :])
    return {
        "perm": perm,
        "windows": tuple(windows),
        "offs": offs,
        "SCN": scn.astype(np.float32),
    }


def _build(windows, offs_total):
    import concourse.mybir as mybir
    import concourse.tile as tile
    from concourse import bacc
    from concourse.bass import ds

    SCAN_OP = _register_scan_op()
    f32 = mybir.dt.float32
    bf16 = mybir.dt.bfloat16
    R = int(os.environ.get("DIAG_R", "6"))

    nc = bacc.Bacc("TRN2", target_bir_lowering=False, debug=False, num_devices=NCORES)
    # X block layout: partition p (= i within chunk), line [ic, b, tau] (4KB bf16)
    X = nc.dram_tensor("X", [NBLK, 128, NI * TB * BC], bf16, kind="ExternalInput").ap()
    # WIHT line: [ic, h]; WHOT line: [g, o] (chunk-major columns)
    WIHT = nc.dram_tensor("WIHT", [128, NI * H], bf16, kind="ExternalInput").ap()
    WHOT = nc.dram_tensor("WHOT", [128, NCH * O], bf16, kind="ExternalInput").ap()
    SCN = nc.dram_tensor("SCN", [128, offs_total], f32, kind="ExternalInput").ap()
    BIAS = nc.dram_tensor("BIAS", [BC, O], f32, kind="ExternalInput").ap()
    Y = nc.dram_tensor("Y", [BC, O], f32, kind="ExternalOutput").ap()

    offs = np.concatenate([[0], np.cumsum(windows)]).astype(int)

    with tile.TileContext(nc) as tc:
        with ExitStack() as ctx:
            consts = ctx.enter_context(tc.tile_pool(name="consts", bufs=1))
            xpool = ctx.enter_context(tc.tile_pool(name="xt", bufs=1))
            ubig = ctx.enter_context(tc.tile_pool(name="ubig", bufs=1))
            usmall = ctx.enter_context(tc.tile_pool(name="usmall", bufs=4))
            ypool = ctx.enter_context(tc.tile_pool(name="yout", bufs=1))
            gpool = ctx.enter_context(tc.tile_pool(name="gpsum", bufs=7, space="PSUM"))
            fpool = ctx.enter_context(tc.tile_pool(name="fpsum", bufs=1, space="PSUM"))

            # --- constants / inputs ---
            wiht_t = consts.tile([128, NI * H], bf16, tag="wiht", name="wiht_t")
            nc.sync.dma_start(wiht_t[:], WIHT)
            scn_t = consts.tile([128, offs_total], f32, tag="scn", name="scn_t")
            nc.sync.dma_start(scn_t[:], SCN)
            xt = [
                xpool.tile([128, NI * TB * BC], bf16, tag=f"x{kb}", name=f"x_{kb}")
                for kb in range(NBLK)
            ]
            # spread X descriptor generation across idle engine queues
            dma_engines = [nc.sync, nc.gpsimd, nc.scalar]
            for kb in range(NBLK):
                dma_engines[kb % 3].dma_start(xt[kb][:], X[kb])
            bias_t = ypool.tile([BC, O], f32, tag="bias", name="bias_t")
            nc.gpsimd.dma_start(bias_t[:], BIAS)
            whot_t = consts.tile([128, NCH * O], bf16, tag="whot", name="whot_t")
            nc.gpsimd.dma_start(whot_t[:], WHOT)

            h_all = consts.tile([128, NCH * BC], bf16, tag="hall", name="h_all")

            # --- chunk-major pipeline: GEMM (PE) -> copy (ACT) -> scan (DVE) ---
            for g in range(NCH):
                kg = windows[g]
                nbg = kg // TB
                fb = NBLK - nbg
                if g == 0:
                    u_t = ubig.tile([128, BC * kg], f32, tag="u0", name="u_g0")
                else:
                    u_t = usmall.tile(
                        [128, BC * USMALL_W], f32, tag="us", name=f"u_g{g}"
                    )
                uwin = u_t[:, ds(0, BC * kg)]
                for rs in range(fb, NBLK, R):
                    run = list(range(rs, min(rs + R, NBLK)))
                    ps = {
                        kb: gpool.tile([128, TB * BC], f32, tag="gp", name=f"gp_{g}_{kb}")
                        for kb in run
                    }
                    for ic in range(NI):
                        for kb in run:
                            nc.tensor.matmul(
                                ps[kb][:],
                                wiht_t[:, ds(ic * H + g * 128, 128)],
                                xt[kb][:, ds(ic * TB * BC, TB * BC)],
                                start=(ic == 0),
                                stop=(ic == NI - 1),
                            )
                    for kb in run:
                        j = kb - fb
                        dst = uwin.rearrange("p (b t) -> p b t", b=BC)[
                            :, :, ds(j * TB, TB)
                        ]
                        src = ps[kb][:].rearrange("p (b t) -> p b t", b=BC)
                        nc.scalar.copy(dst, src)
                # scans: one instruction per batch row, whole window
                scn_g = scn_t[:, ds(int(offs[g]), kg)]
                for b in range(BC):
                    ap = u_t[:, ds(b * kg, kg)]
                    nc.vector._custom_dve(SCAN_OP, out=ap, in0=ap, in1=scn_g)
                # h_final = last scan element per (lane, b) -> bf16
                hsrc = uwin.rearrange("p (b t) -> p b t", b=BC)[:, :, kg - 1]
                nc.vector.tensor_copy(h_all[:, ds(g * BC, BC)], hsrc)

            # --- final projection: Y = h^T @ WHOT + bias ---
            psy = fpool.tile([BC, O], f32, tag="fy", name="psy")
            for g in range(NCH):
                nc.tensor.matmul(
                    psy[:],
                    h_all[:, ds(g * BC, BC)],
                    whot_t[:, ds(g * O, O)],
                    start=(g == 0),
                    stop=(g == NCH - 1),
                )
            y_t = ypool.tile([BC, O], f32, tag="y", name="y_t")
            nc.vector.tensor_tensor(y_t[:], psy[:], bias_t[:], mybir.AluOpType.add)
            nc.sync.dma_start(Y, y_t[:])
    nc.compile()
    return nc


def _get_program(windows, offs_total):
    key = (
        windows,
        os.environ.get("DIAG_R"),
        os.environ.get("DIAG_LN"),
    )
    if key not in _CACHE:
        _CACHE[key] = _build(windows, offs_total)
    return _CACHE[key]


def _ensure_ntff_hook():
    """Provide antenv.axon_hooks (absent in this image) so trace=True works."""
    import sys
    import types

    if "antenv.axon_hooks" in sys.modules:
        return True
    try:
        import antenv

        mod = types.ModuleType("antenv.axon_hooks")
        mod._hook = None

        def set_axon_ntff_profile_hook(h):
            mod._hook = h

        def get_axon_ntff_profile_hook():
            return mod._hook

        mod.set_axon_ntff_profile_hook = set_axon_ntff_profile_hook
        mod.get_axon_ntff_profile_hook = get_axon_ntff_profile_hook
        sys.modules["antenv.axon_hooks"] = mod
        antenv.axon_hooks = mod

        from trn_agent_boot.trn_boot import _ntff_profile_via_ctypes

        hook = _ntff_profile_via_ctypes("/opt/axon/libaxon_pjrt.so")
        mod.set_axon_ntff_profile_hook(hook)
        return hook is not None
    except Exception:
        return False


def kernel(X, W_ih, hh, W_ho, b_ho):
    import ml_dtypes

    from concourse import bass_utils

    X = np.asarray(X, dtype=np.float32)
    W_ih = np.asarray(W_ih, dtype=np.float32)
    hh = np.asarray(hh, dtype=np.float32)
    W_ho = np.asarray(W_ho, dtype=np.float32)
    b_ho = np.asarray(b_ho, dtype=np.float32)

    plan = _make_plan(hh)
    perm = plan["perm"]
    nc = _get_program(plan["windows"], int(plan["offs"][-1]))

    bf = ml_dtypes.bfloat16
    # WIHT [128, NI*H]: line p = [ic, h] with value W_ih[h, i=ic*128+p]
    wiht = np.ascontiguousarray(
        W_ih[perm].T.reshape(NI, 128, H).transpose(1, 0, 2).reshape(128, NI * H)
    ).astype(bf)
    # WHOT [128, NCH*O]: line p = [g, o] with value W_ho[o, h=g*128+p]
    whot = np.ascontiguousarray(
        W_ho[:, perm].T.reshape(NCH, 128, O).transpose(1, 0, 2).reshape(128, NCH * O)
    ).astype(bf)
    bias = np.tile(b_ho[None, :], (BC, 1)).astype(np.float32)

    common = {
        "WIHT": wiht,
        "WHOT": whot,
        "BIAS": bias,
        "SCN": plan["SCN"],
    }
    in_maps = []
    for m in range(NCORES):
        im = dict(common)
        xm = X[:, m * BC : (m + 1) * BC, :]  # [S, BC, I]
        # device layout [NBLK, 128(i-in-chunk), (ic, b, tau)]
        xt = xm.transpose(2, 1, 0).reshape(NI, 128, BC, NBLK, TB)
        xt = np.ascontiguousarray(xt.transpose(3, 1, 0, 2, 4)).reshape(
            NBLK, 128, NI * BC * TB
        )
        im["X"] = xt.astype(bf)
        in_maps.append(im)

    trace = bool(int(os.environ.get("DIAG_TRACE", "0")))
    if trace:
        trace = _ensure_ntff_hook()
    res = None
    for attempt in range(3):
        try:
            res = bass_utils.run_bass_kernel_spmd(
                nc,
                in_maps,
                core_ids=list(range(NCORES)),
                trace=trace,
                tmpdir=os.environ.get("DIAG_TRACE_DIR") or None,
            )
            break
        except Exception:
            if attempt == 2:
                raise
            trace = False  # retry without profiling
    if res.exec_time_ns is not None:
        kernel.last_exec_time_ns = res.exec_time_ns
        kernel.last_mean_exec_time_ns = res.mean_exec_time_ns
    Yfull = np.concatenate([r["Y"] for r in res.results], axis=0)
    return Yfull


kernel.last_exec_time_ns = None
kernel.last_mean_exec_time_ns = None


# revision 15
# speedup vs baseline: 3.6260x; 1.1291x over previous
"""Trainium2 Bass kernel for nn_Diagnet (S=1024, B=64, I=512, H=2048, O=512).

    u = einsum('sbi,hi->sbh', X, W_ih)
    h_t = |u_t + hh * h_{t-1}|   (scan over S, only final h needed)
    Y = h_final @ W_ho.T + b_ho

Strategy (8 NeuronCores, data-parallel over batch, BC=8 rows per core):

* H lanes are permuted so hh is sorted descending and split into 16
  chunks of 128.  A chunk whose largest decay a satisfies a^K < tol
  only needs the last K steps (exact to ~tol relative), so each chunk
  gets a window K_g (multiple of 64), and the GEMM + scan skip
  everything earlier.
* The recurrence is computed by a custom DVE instruction that folds
  the WHOLE window in one go: out[tau] = |out[tau-1] - u[tau]*scn[tau]|
  via scan(ABSOLUTE_DIFF, Src0*Src1).  The running state lives in the
  engine (no SBUF round-trip per step), so the serial chain that
  dominated the naive per-step implementation (~200ns x 1024 steps)
  collapses to one ~K-cycle streaming instruction per (chunk, batch).
  scn[tau] = -a_lane^(K-1-tau) folds the per-step decay multiply into
  a prescale (a>=0 lets a*|x| = |a x|), and the minus sign turns
  ABSOLUTE_DIFF into abs-add.  h_final = last scan element (scale 1).
* GEMM runs in bf16 (1 cycle/row on the PE vs 4 for fp32; X DMA
  halves).  X is host-tiled to [block, i-chunk, 128i, (b,tau)] and
  kept resident in SBUF; the GEMM iterates chunk-major (longest
  window first) so each chunk's scan overlaps later chunks' GEMMs,
  with i-chunk-outer PSUM accumulation runs to amortize LDWEIGHTS.
* The Activation engine drains PSUM -> SBUF with a pure layout copy
  (to b-major contiguous windows); GPSIMD extracts h_final columns
  (cast to bf16); the final projection is 16 accumulating bf16
  matmuls + bias add at the end.
"""

import math
import os

from contextlib import ExitStack

import numpy as np

S, B, I, H, O = 1024, 64, 512, 2048, 512
NCORES = 8
BC = B // NCORES  # 8 batch rows per core
TB = 64  # block granularity for truncation windows
NBLK = S // TB  # 16
NCH = H // 128  # 16 h-chunks
NI = I // 128  # 4 i-chunks
USMALL_W = 256  # max window (cols) for chunks g>=1; K_1 <= 256 needs LN <= ~16

_CACHE = {}


def _register_scan_op():
    import concourse.dve_ops as dve_ops
    from concourse.dve_spec import Spec, Src0, Src1, Zero, scan, lower, AluOp
    from concourse.dve_uop import DveOpSpec

    for op in dve_ops.OPS:
        if op.name == "ABSDIFF_SCALE_SCAN_ANT":
            return op

    def _ref(in0, in1, s0, s1, imm2):
        x = in0.astype(np.float32) * in1.astype(np.float32)
        out = np.empty_like(x)
        m = np.zeros(x.shape[0], np.float32)
        for t in range(x.shape[1]):
            m = np.abs(m - x[:, t])
            out[:, t] = m
        return out

    spec = Spec(
        body=scan(AluOp.ABSOLUTE_DIFF, Src0 * Src1, init=Zero),
        reference=_ref,
    )
    row = max(dve_ops._SUB_OPCODE_FOR_NAME.values()) + 1
    assert row < 0x20
    shas = {}
    for ver in ("v3", "v4"):
        s = DveOpSpec(
            name="ABSDIFF_SCALE_SCAN_ANT", opcode=row, uops=lower(spec, ver=ver), rd1_en=True
        )
        shas[ver] = s.sha(ver)
    op = dve_ops.DveOp("ABSDIFF_SCALE_SCAN_ANT", spec, subdim=False, uops_sha=shas)
    dve_ops._SUB_OPCODE_FOR_NAME["ABSDIFF_SCALE_SCAN_ANT"] = row
    dve_ops.OPS.append(op)
    dve_ops.CUSTOM_DVE_SPECS["ABSDIFF_SCALE_SCAN_ANT"] = spec
    return op


def _make_plan(hh):
    ln = float(os.environ.get("DIAG_LN", "9.2"))  # a^K <= e^-ln truncation tol
    a = np.maximum(np.abs(hh.astype(np.float64)), 1e-30)
    perm = np.argsort(-a, kind="stable")
    ag = a[perm].reshape(NCH, 128)  # [chunk, lane], descending
    windows = []
    for g in range(NCH):
        amax = ag[g, 0]
        if S * math.log(amax) >= -ln:
            kg = S
        else:
            kg = int(math.ceil(ln / math.log(1.0 / amax)))
        kg = min(S, max(TB, ((kg + TB - 1) // TB) * TB))
        windows.append(kg)
    assert all(windows[g] >= windows[g + 1] for g in range(NCH - 1)), windows
    assert all(k <= USMALL_W for k in windows[1:]), (windows, "raise USMALL_W")
    offs = np.concatenate([[0], np.cumsum(windows)]).astype(int)
    scn = np.zeros((128, offs[-1]), dtype=np.float64)
    for g in range(NCH):
        kg = windows[g]
        tau = np.arange(kg)
        scn[:, offs[g] : offs[g] + kg] = -(ag[g][:, None] ** (kg - 1 - tau)[None, :])
    return {
        "perm": perm,
        "windows": tuple(windows),
        "offs": offs,
        "SCN": scn.astype(np.float32),
    }


def _build(windows, offs_total):
    import concourse.mybir as mybir
    import concourse.tile as tile
    from concourse import bacc
    from concourse.bass import ds

    SCAN_OP = _register_scan_op()
    f32 = mybir.dt.float32
    bf16 = mybir.dt.bfloat16
    R = int(os.environ.get("DIAG_R", "6"))

    nc = bacc.Bacc("TRN2", target_bir_lowering=False, debug=False, num_devices=NCORES)
    # X block layout: partition p (= i within chunk), line [ic, b, tau] (4KB bf16)
    X = nc.dram_tensor("X", [NBLK, 128, NI * TB * BC], bf16, kind="ExternalInput").ap()
    # WIHT line: [g, ic, hsub] (per-chunk contiguous pieces); WHOT line: [g, o]
    WIHT = nc.dram_tensor("WIHT", [128, NCH * NI * 128], bf16, kind="ExternalInput").ap()
    WHOT = nc.dram_tensor("WHOT", [128, NCH * O], bf16, kind="ExternalInput").ap()
    SCN = nc.dram_tensor("SCN", [128, offs_total], f32, kind="ExternalInput").ap()
    BIAS = nc.dram_tensor("BIAS", [BC, O], f32, kind="ExternalInput").ap()
    Y = nc.dram_tensor("Y", [BC, O], f32, kind="ExternalOutput").ap()

    offs = np.concatenate([[0], np.cumsum(windows)]).astype(int)

    with tile.TileContext(nc) as tc:
        with ExitStack() as ctx:
            consts = ctx.enter_context(tc.tile_pool(name="consts", bufs=1))
            xpool = ctx.enter_context(tc.tile_pool(name="xt", bufs=1))
            ubig = ctx.enter_context(tc.tile_pool(name="ubig", bufs=1))
            usmall = ctx.enter_context(tc.tile_pool(name="usmall", bufs=4))
            ypool = ctx.enter_context(tc.tile_pool(name="yout", bufs=1))
            gpool = ctx.enter_context(tc.tile_pool(name="gpsum", bufs=7, space="PSUM"))
            fpool = ctx.enter_context(tc.tile_pool(name="fpsum", bufs=1, space="PSUM"))

            # --- inputs; consumption order is chunk 15..1, then 0, so X blocks
            # and per-chunk WIHT pieces are fetched in reverse block order on
            # the two HWDGE queues (sync + scalar) ---
            scn_t = consts.tile([128, offs_total], f32, tag="scn", name="scn_t")
            nc.sync.dma_start(scn_t[:], SCN)
            wiht_t = consts.tile([128, NCH * NI * 128], bf16, tag="wiht", name="wiht_t")
            for g in range(NCH - 1, -1, -1):
                nc.sync.dma_start(
                    wiht_t[:, ds(g * NI * 128, NI * 128)], WIHT[:, ds(g * NI * 128, NI * 128)]
                )
            xt = [
                xpool.tile([128, NI * TB * BC], bf16, tag=f"x{kb}", name=f"x_{kb}")
                for kb in range(NBLK)
            ]
            for kb in range(NBLK - 1, -1, -1):
                nc.scalar.dma_start(xt[kb][:], X[kb])
            bias_t = ypool.tile([BC, O], f32, tag="bias", name="bias_t")
            nc.sync.dma_start(bias_t[:], BIAS)
            whot_t = consts.tile([128, NCH * O], bf16, tag="whot", name="whot_t")
            nc.scalar.dma_start(whot_t[:], WHOT)

            h_all = consts.tile([128, NCH * BC], bf16, tag="hall", name="h_all")

            # --- chunk-major pipeline: GEMM (PE) -> copy (ACT) -> scan (DVE) ---
            chunk_order = list(range(NCH - 1, 0, -1)) + [0]
            for g in chunk_order:
                kg = windows[g]
                nbg = kg // TB
                fb = NBLK - nbg
                if g == 0:
                    u_t = ubig.tile([128, BC * kg], f32, tag="u0", name="u_g0")
                else:
                    u_t = usmall.tile(
                        [128, BC * USMALL_W], f32, tag="us", name=f"u_g{g}"
                    )
                uwin = u_t[:, ds(0, BC * kg)]
                blocks = list(range(NBLK - 1, fb - 1, -1))  # newest X first
                for rs in range(0, len(blocks), R):
                    run = blocks[rs : rs + R]
                    ps = {
                        kb: gpool.tile([128, TB * BC], f32, tag="gp", name=f"gp_{g}_{kb}")
                        for kb in run
                    }
                    for ic in range(NI):
                        for kb in run:
                            nc.tensor.matmul(
                                ps[kb][:],
                                wiht_t[:, ds(g * NI * 128 + ic * 128, 128)],
                                xt[kb][:, ds(ic * TB * BC, TB * BC)],
                                start=(ic == 0),
                                stop=(ic == NI - 1),
                            )
                    for kb in run:
                        j = kb - fb
                        dst = uwin.rearrange("p (b t) -> p b t", b=BC)[
                            :, :, ds(j * TB, TB)
                        ]
                        src = ps[kb][:].rearrange("p (b t) -> p b t", b=BC)
                        nc.scalar.copy(dst, src)
                # scans: one instruction per batch row, whole window
                scn_g = scn_t[:, ds(int(offs[g]), kg)]
                for b in range(BC):
                    ap = u_t[:, ds(b * kg, kg)]
                    nc.vector._custom_dve(SCAN_OP, out=ap, in0=ap, in1=scn_g)
                # h_final = last scan element per (lane, b) -> bf16
                hsrc = uwin.rearrange("p (b t) -> p b t", b=BC)[:, :, kg - 1]
                nc.vector.tensor_copy(h_all[:, ds(g * BC, BC)], hsrc)

            # --- final projection: Y = h^T @ WHOT + bias ---
            # (emitted after all main-GEMM matmuls so no PE-FIFO stall; chunk 0
            # last, so the tail after its scan is a single matmul)
            psy = fpool.tile([BC, O], f32, tag="fy", name="psy")
            for i, g in enumerate(chunk_order):
                nc.tensor.matmul(
                    psy[:],
                    h_all[:, ds(g * BC, BC)],
                    whot_t[:, ds(g * O, O)],
                    start=(i == 0),
                    stop=(i == NCH - 1),
                )
            y_t = ypool.tile([BC, O], f32, tag="y", name="y_t")
            nc.vector.tensor_tensor(y_t[:], psy[:], bias_t[:], mybir.AluOpType.add)
            nc.sync.dma_start(Y, y_t[:])
    nc.compile()
    return nc


def _get_program(windows, offs_total):
    key = (
        windows,
        os.environ.get("DIAG_R"),
        os.environ.get("DIAG_LN"),
    )
    if key not in _CACHE:
        _CACHE[key] = _build(windows, offs_total)
    return _CACHE[key]


def _ensure_ntff_hook():
    """Provide antenv.axon_hooks (absent in this image) so trace=True works."""
    import sys
    import types

    if "antenv.axon_hooks" in sys.modules:
        return True
    try:
        import antenv

        mod = types.ModuleType("antenv.axon_hooks")
        mod._hook = None

        def set_axon_ntff_profile_hook(h):
            mod._hook = h

        def get_axon_ntff_profile_hook():
            return mod._hook

        mod.set_axon_ntff_profile_hook = set_axon_ntff_profile_hook
        mod.get_axon_ntff_profile_hook = get_axon_ntff_profile_hook
        sys.modules["antenv.axon_hooks"] = mod
        antenv.axon_hooks = mod

        from trn_agent_boot.trn_boot import _ntff_profile_via_ctypes

        hook = _ntff_profile_via_ctypes("/opt/axon/libaxon_pjrt.so")
        mod.set_axon_ntff_profile_hook(hook)
        return hook is not None
    except Exception:
        return False


def kernel(X, W_ih, hh, W_ho, b_ho):
    import ml_dtypes

    from concourse import bass_utils

    X = np.asarray(X, dtype=np.float32)
    W_ih = np.asarray(W_ih, dtype=np.float32)
    hh = np.asarray(hh, dtype=np.float32)
    W_ho = np.asarray(W_ho, dtype=np.float32)
    b_ho = np.asarray(b_ho, dtype=np.float32)

    plan = _make_plan(hh)
    perm = plan["perm"]
    nc = _get_program(plan["windows"], int(plan["offs"][-1]))

    bf = ml_dtypes.bfloat16
    # WIHT [128, NCH*NI*128]: line p = [g, ic, hsub], W_ih[h=g*128+hsub, i=ic*128+p]
    wiht = np.ascontiguousarray(
        W_ih[perm].T.reshape(NI, 128, NCH, 128).transpose(1, 2, 0, 3).reshape(128, -1)
    ).astype(bf)
    # WHOT [128, NCH*O]: line p = [g, o] with value W_ho[o, h=g*128+p]
    whot = np.ascontiguousarray(
        W_ho[:, perm].T.reshape(NCH, 128, O).transpose(1, 0, 2).reshape(128, NCH * O)
    ).astype(bf)
    bias = np.tile(b_ho[None, :], (BC, 1)).astype(np.float32)

    common = {
        "WIHT": wiht,
        "WHOT": whot,
        "BIAS": bias,
        "SCN": plan["SCN"],
    }
    in_maps = []
    for m in range(NCORES):
        im = dict(common)
        xm = X[:, m * BC : (m + 1) * BC, :]  # [S, BC, I]
        # device layout [NBLK, 128(i-in-chunk), (ic, b, tau)]
        xt = xm.transpose(2, 1, 0).reshape(NI, 128, BC, NBLK, TB)
        xt = np.ascontiguousarray(xt.transpose(3, 1, 0, 2, 4)).reshape(
            NBLK, 128, NI * BC * TB
        )
        im["X"] = xt.astype(bf)
        in_maps.append(im)

    trace = bool(int(os.environ.get("DIAG_TRACE", "0")))
    if trace:
        trace = _ensure_ntff_hook()
    res = None
    for attempt in range(3):
        try:
            res = bass_utils.run_bass_kernel_spmd(
                nc,
                in_maps,
                core_ids=list(range(NCORES)),
                trace=trace,
                tmpdir=os.environ.get("DIAG_TRACE_DIR") or None,
            )
            break
        except Exception:
            if attempt == 2:
                raise
            trace = False  # retry without profiling
    if res.exec_time_ns is not None:
        kernel.last_exec_time_ns = res.exec_time_ns
        kernel.last_mean_exec_time_ns = res.mean_exec_time_ns
    Yfull = np.concatenate([r["Y"] for r in res.results], axis=0)
    return Yfull


kernel.last_exec_time_ns = None
kernel.last_mean_exec_time_ns = None


# revision 23
# speedup vs baseline: 4.3250x; 1.1928x over previous
"""Trainium2 Bass kernel for nn_Diagnet (S=1024, B=64, I=512, H=2048, O=512).

    u = einsum('sbi,hi->sbh', X, W_ih)
    h_t = |u_t + hh * h_{t-1}|   (scan over S, only final h needed)
    Y = h_final @ W_ho.T + b_ho

Strategy (8 NeuronCores, data-parallel over batch, BC=8 rows per core):

* H lanes are permuted so hh is sorted descending and split into 16
  chunks of 128.  A chunk whose largest decay a satisfies a^K < tol
  only needs the last K steps (exact to ~tol relative), so each chunk
  gets a window K_g (multiple of 64), and the GEMM + scan skip
  everything earlier.
* The recurrence is computed by a custom DVE instruction that folds
  the WHOLE window in one go: out[tau] = |out[tau-1] - u[tau]*scn[tau]|
  via scan(ABSOLUTE_DIFF, Src0*Src1).  The running state lives in the
  engine (no SBUF round-trip per step), so the serial chain that
  dominated the naive per-step implementation (~200ns x 1024 steps)
  collapses to one ~K-cycle streaming instruction per (chunk, batch).
  scn[tau] = -a_lane^(K-1-tau) folds the per-step decay multiply into
  a prescale (a>=0 lets a*|x| = |a x|), and the minus sign turns
  ABSOLUTE_DIFF into abs-add.  h_final = last scan element (scale 1).
* GEMM runs in bf16 (1 cycle/row on the PE vs 4 for fp32; X DMA
  halves).  X is host-tiled to [block, i-chunk, 128i, (b,tau)] and
  kept resident in SBUF; the GEMM iterates chunk-major (longest
  window first) so each chunk's scan overlaps later chunks' GEMMs,
  with i-chunk-outer PSUM accumulation runs to amortize LDWEIGHTS.
* The Activation engine drains PSUM -> SBUF with a pure layout copy
  (to b-major contiguous windows); GPSIMD extracts h_final columns
  (cast to bf16); the final projection is 16 accumulating bf16
  matmuls + bias add at the end.
"""

import math
import os

from contextlib import ExitStack

import numpy as np

S, B, I, H, O = 1024, 64, 512, 2048, 512
NCORES = 8
BC = B // NCORES  # 8 batch rows per core
TB = 64  # block granularity for truncation windows
NBLK = S // TB  # 16
NCH = H // 128  # 16 h-chunks
NI = I // 128  # 4 i-chunks
USMALL_W = 256  # max window (cols) for chunks g>=1; K_1 <= 256 needs LN <= ~16

_CACHE = {}


def _register_scan_ops():
    """Two fold ops: m[t] = |m[t-1] - in0[t]*in1[t]|, seeded with 0 or with a
    per-partition value (s0) for chaining segment scans."""
    import concourse.dve_ops as dve_ops
    from concourse.dve_spec import C0, Spec, Src0, Src1, Zero, scan, lower, AluOp
    from concourse.dve_uop import DveOpSpec

    have = {op.name: op for op in dve_ops.OPS}
    if "ABSDIFF_SCALE_SCAN_ANT" in have:
        return have["ABSDIFF_SCALE_SCAN_ANT"], have["ABSDIFF_SCALE_SCAN_SEED_ANT"]

    def _ref_factory(seeded):
        def _ref(in0, in1, s0, s1, imm2):
            x = in0.astype(np.float32) * in1.astype(np.float32)
            out = np.empty_like(x)
            m = (
                np.broadcast_to(np.asarray(s0, np.float32).reshape(-1), (x.shape[0],))
                if seeded
                else np.zeros(x.shape[0], np.float32)
            ).copy()
            for t in range(x.shape[1]):
                m = np.abs(m - x[:, t])
                out[:, t] = m
            return out

        return _ref

    ops = []
    for name, init, seeded in (
        ("ABSDIFF_SCALE_SCAN_ANT", Zero, False),
        ("ABSDIFF_SCALE_SCAN_SEED_ANT", C0, True),
    ):
        spec = Spec(
            body=scan(AluOp.ABSOLUTE_DIFF, Src0 * Src1, init=init),
            reference=_ref_factory(seeded),
        )
        row = max(dve_ops._SUB_OPCODE_FOR_NAME.values()) + 1
        assert row < 0x20
        shas = {}
        for ver in ("v3", "v4"):
            s = DveOpSpec(name=name, opcode=row, uops=lower(spec, ver=ver), rd1_en=True)
            shas[ver] = s.sha(ver)
        op = dve_ops.DveOp(name, spec, subdim=False, uops_sha=shas)
        dve_ops._SUB_OPCODE_FOR_NAME[name] = row
        dve_ops.OPS.append(op)
        dve_ops.CUSTOM_DVE_SPECS[name] = spec
        ops.append(op)
    return ops[0], ops[1]


def _make_plan(hh):
    ln = float(os.environ.get("DIAG_LN", "9.2"))  # a^K <= e^-ln truncation tol
    a = np.maximum(np.abs(hh.astype(np.float64)), 1e-30)
    perm = np.argsort(-a, kind="stable")
    ag = a[perm].reshape(NCH, 128)  # [chunk, lane], descending
    windows = []
    for g in range(NCH):
        amax = ag[g, 0]
        if S * math.log(amax) >= -ln:
            kg = S
        else:
            kg = int(math.ceil(ln / math.log(1.0 / amax)))
        kg = min(S, max(TB, ((kg + TB - 1) // TB) * TB))
        windows.append(kg)
    assert all(windows[g] >= windows[g + 1] for g in range(NCH - 1)), windows
    assert all(k <= USMALL_W for k in windows[1:]), (windows, "raise USMALL_W")
    offs = np.concatenate([[0], np.cumsum(windows)]).astype(int)
    scn = np.zeros((128, offs[-1]), dtype=np.float64)
    for g in range(NCH):
        kg = windows[g]
        tau = np.arange(kg)
        scn[:, offs[g] : offs[g] + kg] = -(ag[g][:, None] ** (kg - 1 - tau)[None, :])
    return {
        "perm": perm,
        "windows": tuple(windows),
        "offs": offs,
        "SCN": scn,  # float64; cast at the call site
    }


def _build(windows, offs_total):
    import concourse.mybir as mybir
    import concourse.tile as tile
    from concourse import bacc
    from concourse.bass import ds

    SCAN_OP, SCAN_SEED_OP = _register_scan_ops()
    f32 = mybir.dt.float32
    bf16 = mybir.dt.bfloat16
    R = int(os.environ.get("DIAG_R", "6"))

    nc = bacc.Bacc("TRN2", target_bir_lowering=False, debug=False, num_devices=NCORES)
    # X block layout: partition p (= i within chunk), line [ic, b, tau] (4KB bf16)
    X = nc.dram_tensor("X", [NBLK, 128, NI * TB * BC], bf16, kind="ExternalInput").ap()
    # WIHT line: [g, ic, hsub] (per-chunk contiguous pieces); WHOT line: [g, o]
    WIHT = nc.dram_tensor("WIHT", [128, NCH * NI * 128], bf16, kind="ExternalInput").ap()
    WHOT = nc.dram_tensor("WHOT", [128, NCH * O], bf16, kind="ExternalInput").ap()
    SCN = nc.dram_tensor("SCN", [128, offs_total], bf16, kind="ExternalInput").ap()
    BIAS = nc.dram_tensor("BIAS", [BC, O], f32, kind="ExternalInput").ap()
    Y = nc.dram_tensor("Y", [BC, O], f32, kind="ExternalOutput").ap()

    offs = np.concatenate([[0], np.cumsum(windows)]).astype(int)

    with tile.TileContext(nc) as tc:
        with ExitStack() as ctx:
            consts = ctx.enter_context(tc.tile_pool(name="consts", bufs=1))
            xpool = ctx.enter_context(tc.tile_pool(name="xt", bufs=1))
            ubig = ctx.enter_context(tc.tile_pool(name="ubig", bufs=1))
            usmall = ctx.enter_context(tc.tile_pool(name="usmall", bufs=6))
            ypool = ctx.enter_context(tc.tile_pool(name="yout", bufs=1))
            gpool = ctx.enter_context(tc.tile_pool(name="gpsum", bufs=7, space="PSUM"))
            fpool = ctx.enter_context(tc.tile_pool(name="fpsum", bufs=1, space="PSUM"))

            # --- inputs.  Consumption order: chunks 15..1 (need only the last
            # 1-3 X blocks + their WIHT pieces), then chunk 0 which scans
            # blocks 0..15 in ascending time order.  X arrival order matches:
            # 15,14,13 first, then 0,1,2,...,12, split across the two HWDGE
            # queues (sync + scalar). ---
            wiht_t = consts.tile([128, NCH * NI * 128], bf16, tag="wiht", name="wiht_t")
            scn_t = consts.tile([128, offs_total], bf16, tag="scn", name="scn_t")
            xt = [
                xpool.tile([128, NI * TB * BC], bf16, tag=f"x{kb}", name=f"x_{kb}")
                for kb in range(NBLK)
            ]
            for g in range(NCH - 1, NCH - 4, -1):
                nc.sync.dma_start(
                    wiht_t[:, ds(g * NI * 128, NI * 128)],
                    WIHT[:, ds(g * NI * 128, NI * 128)],
                )
            nc.scalar.dma_start(xt[NBLK - 1][:], X[NBLK - 1])
            nc.scalar.dma_start(xt[NBLK - 2][:], X[NBLK - 2])
            nc.scalar.dma_start(xt[NBLK - 3][:], X[NBLK - 3])
            for g in range(NCH - 4, -1, -1):
                nc.sync.dma_start(
                    wiht_t[:, ds(g * NI * 128, NI * 128)],
                    WIHT[:, ds(g * NI * 128, NI * 128)],
                )
            nc.sync.dma_start(scn_t[:], SCN)
            # chunk-0 blocks in ascending (scan) order, alternating queues
            for kb in range(0, NBLK - 3):
                (nc.sync if kb % 2 else nc.scalar).dma_start(xt[kb][:], X[kb])
            bias_t = ypool.tile([BC, O], f32, tag="bias", name="bias_t")
            nc.sync.dma_start(bias_t[:], BIAS)
            whot_t = consts.tile([128, NCH * O], bf16, tag="whot", name="whot_t")
            nc.scalar.dma_start(whot_t[:], WHOT)

            h_all = consts.tile([128, NCH * BC], bf16, tag="hall", name="h_all")

            # --- chunk-major pipeline: GEMM (PE) -> copy (ACT) -> scan (DVE) ---
            chunk_order = list(range(NCH - 1, 0, -1)) + [0]
            for g in chunk_order:
                kg = windows[g]
                nbg = kg // TB
                fb = NBLK - nbg
                if g == 0:
                    u_t = ubig.tile([128, BC * kg], f32, tag="u0", name="u_g0")
                else:
                    u_t = usmall.tile(
                        [128, BC * USMALL_W], f32, tag="us", name=f"u_g{g}"
                    )
                uwin = u_t[:, ds(0, BC * kg)]
                # chunk 0 consumes blocks in ascending (scan) order so each
                # GEMM run's segment scan chains off the previous one; other
                # chunks take newest-first (their X arrives first).
                blocks = (
                    list(range(fb, NBLK))
                    if g == 0
                    else list(range(NBLK - 1, fb - 1, -1))
                )
                for rs in range(0, len(blocks), R):
                    run = blocks[rs : rs + R]
                    ps = {
                        kb: gpool.tile([128, TB * BC], f32, tag="gp", name=f"gp_{g}_{kb}")
                        for kb in run
                    }
                    for ic in range(NI):
                        for kb in run:
                            nc.tensor.matmul(
                                ps[kb][:],
                                wiht_t[:, ds(g * NI * 128 + ic * 128, 128)],
                                xt[kb][:, ds(ic * TB * BC, TB * BC)],
                                start=(ic == 0),
                                stop=(ic == NI - 1),
                            )
                    for kb in run:
                        j = kb - fb
                        dst = uwin.rearrange("p (b t) -> p b t", b=BC)[
                            :, :, ds(j * TB, TB)
                        ]
                        src = ps[kb][:].rearrange("p (b t) -> p b t", b=BC)
                        nc.scalar.copy(dst, src)
                    if g == 0:
                        # segment scan right behind this run, seeded by the
                        # previous segment's last element per (lane, b)
                        seg0 = (run[0] - fb) * TB
                        seg = len(run) * TB
                        scn_s = scn_t[:, ds(int(offs[g]) + seg0, seg)]
                        for b in range(BC):
                            ap = u_t[:, ds(b * kg + seg0, seg)]
                            if rs == 0:
                                nc.vector._custom_dve(SCAN_OP, out=ap, in0=ap, in1=scn_s)
                            else:
                                seed = u_t[:, ds(b * kg + seg0 - 1, 1)]
                                nc.vector._custom_dve(
                                    SCAN_SEED_OP, out=ap, in0=ap, in1=scn_s, s0=seed
                                )
                if g != 0:
                    # whole-window scans: one instruction per batch row
                    scn_g = scn_t[:, ds(int(offs[g]), kg)]
                    for b in range(BC):
                        ap = u_t[:, ds(b * kg, kg)]
                        nc.vector._custom_dve(SCAN_OP, out=ap, in0=ap, in1=scn_g)
                # h_final = last scan element per (lane, b) -> bf16
                hsrc = uwin.rearrange("p (b t) -> p b t", b=BC)[:, :, kg - 1]
                nc.vector.tensor_copy(h_all[:, ds(g * BC, BC)], hsrc)

            # --- final projection: Y = h^T @ WHOT + bias ---
            # (emitted after all main-GEMM matmuls so no PE-FIFO stall; chunk 0
            # last, so the tail after its scan is a single matmul)
            psy = fpool.tile([BC, O], f32, tag="fy", name="psy")
            for i, g in enumerate(chunk_order):
                nc.tensor.matmul(
                    psy[:],
                    h_all[:, ds(g * BC, BC)],
                    whot_t[:, ds(g * O, O)],
                    start=(i == 0),
                    stop=(i == NCH - 1),
                )
            y_t = ypool.tile([BC, O], f32, tag="y", name="y_t")
            nc.vector.tensor_tensor(y_t[:], psy[:], bias_t[:], mybir.AluOpType.add)
            nc.sync.dma_start(Y, y_t[:])
    nc.compile()
    return nc


def _get_program(windows, offs_total):
    key = (
        windows,
        os.environ.get("DIAG_R"),
        os.environ.get("DIAG_LN"),
    )
    if key not in _CACHE:
        _CACHE[key] = _build(windows, offs_total)
    return _CACHE[key]


def _ensure_ntff_hook():
    """Provide antenv.axon_hooks (absent in this image) so trace=True works."""
    import sys
    import types

    if "antenv.axon_hooks" in sys.modules:
        return True
    try:
        import antenv

        mod = types.ModuleType("antenv.axon_hooks")
        mod._hook = None

        def set_axon_ntff_profile_hook(h):
            mod._hook = h

        def get_axon_ntff_profile_hook():
            return mod._hook

        mod.set_axon_ntff_profile_hook = set_axon_ntff_profile_hook
        mod.get_axon_ntff_profile_hook = get_axon_ntff_profile_hook
        sys.modules["antenv.axon_hooks"] = mod
        antenv.axon_hooks = mod

        from trn_agent_boot.trn_boot import _ntff_profile_via_ctypes

        hook = _ntff_profile_via_ctypes("/opt/axon/libaxon_pjrt.so")
        mod.set_axon_ntff_profile_hook(hook)
        return hook is not None
    except Exception:
        return False


def kernel(X, W_ih, hh, W_ho, b_ho):
    import ml_dtypes

    from concourse import bass_utils

    X = np.asarray(X, dtype=np.float32)
    W_ih = np.asarray(W_ih, dtype=np.float32)
    hh = np.asarray(hh, dtype=np.float32)
    W_ho = np.asarray(W_ho, dtype=np.float32)
    b_ho = np.asarray(b_ho, dtype=np.float32)

    plan = _make_plan(hh)
    perm = plan["perm"]
    nc = _get_program(plan["windows"], int(plan["offs"][-1]))

    bf = ml_dtypes.bfloat16
    # WIHT [128, NCH*NI*128]: line p = [g, ic, hsub], W_ih[h=g*128+hsub, i=ic*128+p]
    wiht = np.ascontiguousarray(
        W_ih[perm].T.reshape(NI, 128, NCH, 128).transpose(1, 2, 0, 3).reshape(128, -1)
    ).astype(bf)
    # WHOT [128, NCH*O]: line p = [g, o] with value W_ho[o, h=g*128+p]
    whot = np.ascontiguousarray(
        W_ho[:, perm].T.reshape(NCH, 128, O).transpose(1, 0, 2).reshape(128, NCH * O)
    ).astype(bf)
    bias = np.tile(b_ho[None, :], (BC, 1)).astype(np.float32)

    common = {
        "WIHT": wiht,
        "WHOT": whot,
        "BIAS": bias,
        "SCN": plan["SCN"].astype(bf),
    }
    in_maps = []
    for m in range(NCORES):
        im = dict(common)
        xm = X[:, m * BC : (m + 1) * BC, :]  # [S, BC, I]
        # device layout [NBLK, 128(i-in-chunk), (ic, b, tau)]
        xt = xm.transpose(2, 1, 0).reshape(NI, 128, BC, NBLK, TB)
        xt = np.ascontiguousarray(xt.transpose(3, 1, 0, 2, 4)).reshape(
            NBLK, 128, NI * BC * TB
        )
        im["X"] = xt.astype(bf)
        in_maps.append(im)

    trace = bool(int(os.environ.get("DIAG_TRACE", "0")))
    if trace:
        trace = _ensure_ntff_hook()
    res = None
    for attempt in range(3):
        try:
            res = bass_utils.run_bass_kernel_spmd(
                nc,
                in_maps,
                core_ids=list(range(NCORES)),
                trace=trace,
                tmpdir=os.environ.get("DIAG_TRACE_DIR") or None,
            )
            break
        except Exception:
            if attempt == 2:
                raise
            trace = False  # retry without profiling
    if res.exec_time_ns is not None:
        kernel.last_exec_time_ns = res.exec_time_ns
        kernel.last_mean_exec_time_ns = res.mean_exec_time_ns
    Yfull = np.concatenate([r["Y"] for r in res.results], axis=0)
    return Yfull


kernel.last_exec_time_ns = None
kernel.last_mean_exec_time_ns = None


# revision 26
# speedup vs baseline: 4.6839x; 1.0830x over previous
"""Trainium2 Bass kernel for nn_Diagnet (S=1024, B=64, I=512, H=2048, O=512).

    u = einsum('sbi,hi->sbh', X, W_ih)
    h_t = |u_t + hh * h_{t-1}|   (scan over S, only final h needed)
    Y = h_final @ W_ho.T + b_ho

Strategy (8 NeuronCores, data-parallel over batch, BC=8 rows per core):

* H lanes are permuted so hh is sorted descending and split into 16
  chunks of 128.  A chunk whose largest decay a satisfies a^K < tol
  only needs the last K steps (exact to ~tol relative), so each chunk
  gets a window K_g (multiple of 64), and the GEMM + scan skip
  everything earlier.
* The recurrence is computed by a custom DVE instruction that folds
  the WHOLE window in one go: out[tau] = |out[tau-1] - u[tau]*scn[tau]|
  via scan(ABSOLUTE_DIFF, Src0*Src1).  The running state lives in the
  engine (no SBUF round-trip per step), so the serial chain that
  dominated the naive per-step implementation (~200ns x 1024 steps)
  collapses to one ~K-cycle streaming instruction per (chunk, batch).
  scn[tau] = -a_lane^(K-1-tau) folds the per-step decay multiply into
  a prescale (a>=0 lets a*|x| = |a x|), and the minus sign turns
  ABSOLUTE_DIFF into abs-add.  h_final = last scan element (scale 1).
* GEMM runs in bf16 (1 cycle/row on the PE vs 4 for fp32; X DMA
  halves).  X is host-tiled to [block, i-chunk, 128i, (b,tau)] and
  kept resident in SBUF; the GEMM iterates chunk-major (longest
  window first) so each chunk's scan overlaps later chunks' GEMMs,
  with i-chunk-outer PSUM accumulation runs to amortize LDWEIGHTS.
* The Activation engine drains PSUM -> SBUF with a pure layout copy
  (to b-major contiguous windows); GPSIMD extracts h_final columns
  (cast to bf16); the final projection is 16 accumulating bf16
  matmuls + bias add at the end.
"""

import math
import os

from contextlib import ExitStack

import numpy as np

S, B, I, H, O = 1024, 64, 512, 2048, 512
NCORES = 8
BC = B // NCORES  # 8 batch rows per core
TB = 64  # block granularity for truncation windows
NBLK = S // TB  # 16
NCH = H // 128  # 16 h-chunks
NI = I // 128  # 4 i-chunks
USMALL_W = 256  # max window (cols) for chunks g>=1; K_1 <= 256 needs LN <= ~16

_CACHE = {}


def _register_scan_ops():
    """Two fold ops: m[t] = |m[t-1] - in0[t]*in1[t]|, seeded with 0 or with a
    per-partition value (s0) for chaining segment scans."""
    import concourse.dve_ops as dve_ops
    from concourse.dve_spec import C0, Spec, Src0, Src1, Zero, scan, lower, AluOp
    from concourse.dve_uop import DveOpSpec

    have = {op.name: op for op in dve_ops.OPS}
    if "ABSDIFF_SCALE_SCAN_ANT" in have:
        return have["ABSDIFF_SCALE_SCAN_ANT"], have["ABSDIFF_SCALE_SCAN_SEED_ANT"]

    def _ref_factory(seeded):
        def _ref(in0, in1, s0, s1, imm2):
            x = in0.astype(np.float32) * in1.astype(np.float32)
            out = np.empty_like(x)
            m = (
                np.broadcast_to(np.asarray(s0, np.float32).reshape(-1), (x.shape[0],))
                if seeded
                else np.zeros(x.shape[0], np.float32)
            ).copy()
            for t in range(x.shape[1]):
                m = np.abs(m - x[:, t])
                out[:, t] = m
            return out

        return _ref

    ops = []
    for name, init, seeded in (
        ("ABSDIFF_SCALE_SCAN_ANT", Zero, False),
        ("ABSDIFF_SCALE_SCAN_SEED_ANT", C0, True),
    ):
        spec = Spec(
            body=scan(AluOp.ABSOLUTE_DIFF, Src0 * Src1, init=init),
            reference=_ref_factory(seeded),
        )
        row = max(dve_ops._SUB_OPCODE_FOR_NAME.values()) + 1
        assert row < 0x20
        shas = {}
        for ver in ("v3", "v4"):
            s = DveOpSpec(name=name, opcode=row, uops=lower(spec, ver=ver), rd1_en=True)
            shas[ver] = s.sha(ver)
        op = dve_ops.DveOp(name, spec, subdim=False, uops_sha=shas)
        dve_ops._SUB_OPCODE_FOR_NAME[name] = row
        dve_ops.OPS.append(op)
        dve_ops.CUSTOM_DVE_SPECS[name] = spec
        ops.append(op)
    return ops[0], ops[1]


def _make_plan(hh):
    ln = float(os.environ.get("DIAG_LN", "9.2"))  # a^K <= e^-ln truncation tol
    a = np.maximum(np.abs(hh.astype(np.float64)), 1e-30)
    perm = np.argsort(-a, kind="stable")
    ag = a[perm].reshape(NCH, 128)  # [chunk, lane], descending
    windows = []
    for g in range(NCH):
        amax = ag[g, 0]
        if S * math.log(amax) >= -ln:
            kg = S
        else:
            kg = int(math.ceil(ln / math.log(1.0 / amax)))
        kg = min(S, max(TB, ((kg + TB - 1) // TB) * TB))
        windows.append(kg)
    assert all(windows[g] >= windows[g + 1] for g in range(NCH - 1)), windows
    assert all(k <= USMALL_W for k in windows[1:]), (windows, "raise USMALL_W")
    offs = np.concatenate([[0], np.cumsum(windows)]).astype(int)
    scn = np.zeros((128, offs[-1]), dtype=np.float64)
    for g in range(NCH):
        kg = windows[g]
        tau = np.arange(kg)
        scn[:, offs[g] : offs[g] + kg] = -(ag[g][:, None] ** (kg - 1 - tau)[None, :])
    return {
        "perm": perm,
        "windows": tuple(windows),
        "offs": offs,
        "SCN": scn,  # float64; cast at the call site
    }


def _build(windows, offs_total):
    import concourse.mybir as mybir
    import concourse.tile as tile
    from concourse import bacc
    from concourse.bass import ds

    SCAN_OP, SCAN_SEED_OP = _register_scan_ops()
    f32 = mybir.dt.float32
    bf16 = mybir.dt.bfloat16
    R = int(os.environ.get("DIAG_R", "6"))

    nc = bacc.Bacc("TRN2", target_bir_lowering=False, debug=False, num_devices=NCORES)
    # X block layout: partition p (= i within chunk), line [ic, b, tau] (4KB bf16)
    X = nc.dram_tensor("X", [NBLK, 128, NI * TB * BC], bf16, kind="ExternalInput").ap()
    # WIHT line: [g, ic, hsub] (per-chunk contiguous pieces); WHOT line: [g, o]
    WIHT = nc.dram_tensor("WIHT", [128, NCH * NI * 128], bf16, kind="ExternalInput").ap()
    WHOT = nc.dram_tensor("WHOT", [128, NCH * O], bf16, kind="ExternalInput").ap()
    SCN = nc.dram_tensor("SCN", [128, offs_total], bf16, kind="ExternalInput").ap()
    BIAS = nc.dram_tensor("BIAS", [BC, O], f32, kind="ExternalInput").ap()
    Y = nc.dram_tensor("Y", [BC, O], f32, kind="ExternalOutput").ap()

    offs = np.concatenate([[0], np.cumsum(windows)]).astype(int)

    with tile.TileContext(nc) as tc:
        with ExitStack() as ctx:
            consts = ctx.enter_context(tc.tile_pool(name="consts", bufs=1))
            xpool = ctx.enter_context(tc.tile_pool(name="xt", bufs=1))
            ubig = ctx.enter_context(tc.tile_pool(name="ubig", bufs=1))
            usmall = ctx.enter_context(tc.tile_pool(name="usmall", bufs=6))
            ypool = ctx.enter_context(tc.tile_pool(name="yout", bufs=1))
            gpool = ctx.enter_context(tc.tile_pool(name="gpsum", bufs=7, space="PSUM"))
            fpool = ctx.enter_context(tc.tile_pool(name="fpsum", bufs=1, space="PSUM"))

            # --- inputs.  Consumption order: chunks 15..1 (need only the last
            # 1-3 X blocks + their WIHT pieces), then chunk 0 which scans
            # blocks 0..15 in ascending time order.  X arrival order matches:
            # 15,14,13 first, then 0,1,2,...,12, split across the two HWDGE
            # queues (sync + scalar). ---
            wiht_t = consts.tile([128, NCH * NI * 128], bf16, tag="wiht", name="wiht_t")
            scn_t = consts.tile([128, offs_total], bf16, tag="scn", name="scn_t")
            xt = [
                xpool.tile([128, NI * TB * BC], bf16, tag=f"x{kb}", name=f"x_{kb}")
                for kb in range(NBLK)
            ]
            # all DMA on the SP queue: it has no other work, so descriptor
            # generation never blocks a compute engine's program
            for g in range(NCH - 1, NCH - 4, -1):
                nc.sync.dma_start(
                    wiht_t[:, ds(g * NI * 128, NI * 128)],
                    WIHT[:, ds(g * NI * 128, NI * 128)],
                )
            nc.sync.dma_start(xt[NBLK - 1][:], X[NBLK - 1])
            nc.sync.dma_start(xt[NBLK - 2][:], X[NBLK - 2])
            nc.sync.dma_start(xt[NBLK - 3][:], X[NBLK - 3])
            for g in range(NCH - 4, -1, -1):
                nc.sync.dma_start(
                    wiht_t[:, ds(g * NI * 128, NI * 128)],
                    WIHT[:, ds(g * NI * 128, NI * 128)],
                )
            nc.sync.dma_start(scn_t[:], SCN)
            # chunk-0 blocks in ascending (scan) order
            for kb in range(0, NBLK - 3):
                nc.sync.dma_start(xt[kb][:], X[kb])
            bias_t = ypool.tile([BC, O], f32, tag="bias", name="bias_t")
            nc.sync.dma_start(bias_t[:], BIAS)
            whot_t = consts.tile([128, NCH * O], bf16, tag="whot", name="whot_t")
            nc.sync.dma_start(whot_t[:], WHOT)

            h_all = consts.tile([128, NCH * BC], bf16, tag="hall", name="h_all")

            # --- chunk-major pipeline: GEMM (PE) -> copy (ACT) -> scan (DVE) ---
            chunk_order = list(range(NCH - 1, 0, -1)) + [0]
            for g in chunk_order:
                kg = windows[g]
                nbg = kg // TB
                fb = NBLK - nbg
                if g == 0:
                    u_t = ubig.tile([128, BC * kg], f32, tag="u0", name="u_g0")
                else:
                    u_t = usmall.tile(
                        [128, BC * USMALL_W], f32, tag="us", name=f"u_g{g}"
                    )
                uwin = u_t[:, ds(0, BC * kg)]
                # chunk 0 consumes blocks in ascending (scan) order so each
                # GEMM run's segment scan chains off the previous one; other
                # chunks take newest-first (their X arrives first).
                blocks = (
                    list(range(fb, NBLK))
                    if g == 0
                    else list(range(NBLK - 1, fb - 1, -1))
                )
                if g == 0:
                    # end with a 1-block segment so the post-GEMM scan tail of
                    # the longest chunk is as short as possible
                    sizes = []
                    left = len(blocks)
                    while left > R:
                        sizes.append(R)
                        left -= R
                    sizes += [left - 1, 1] if left > 1 else [1]
                else:
                    sizes = [
                        len(blocks[rs : rs + R]) for rs in range(0, len(blocks), R)
                    ]
                runs = []
                pos = 0
                for sz in sizes:
                    runs.append(blocks[pos : pos + sz])
                    pos += sz
                for run in runs:
                    rs = blocks.index(run[0])
                    ps = {
                        kb: gpool.tile([128, TB * BC], f32, tag="gp", name=f"gp_{g}_{kb}")
                        for kb in run
                    }
                    for ic in range(NI):
                        for kb in run:
                            nc.tensor.matmul(
                                ps[kb][:],
                                wiht_t[:, ds(g * NI * 128 + ic * 128, 128)],
                                xt[kb][:, ds(ic * TB * BC, TB * BC)],
                                start=(ic == 0),
                                stop=(ic == NI - 1),
                            )
                    for kb in run:
                        j = kb - fb
                        dst = uwin.rearrange("p (b t) -> p b t", b=BC)[
                            :, :, ds(j * TB, TB)
                        ]
                        src = ps[kb][:].rearrange("p (b t) -> p b t", b=BC)
                        nc.scalar.copy(dst, src)
                    if g == 0:
                        # segment scan right behind this run, seeded by the
                        # previous segment's last element per (lane, b)
                        seg0 = (run[0] - fb) * TB
                        seg = len(run) * TB
                        scn_s = scn_t[:, ds(int(offs[g]) + seg0, seg)]
                        for b in range(BC):
                            ap = u_t[:, ds(b * kg + seg0, seg)]
                            if rs == 0:
                                nc.vector._custom_dve(SCAN_OP, out=ap, in0=ap, in1=scn_s)
                            else:
                                seed = u_t[:, ds(b * kg + seg0 - 1, 1)]
                                nc.vector._custom_dve(
                                    SCAN_SEED_OP, out=ap, in0=ap, in1=scn_s, s0=seed
                                )
                if g != 0:
                    # whole-window scans: one instruction per batch row
                    scn_g = scn_t[:, ds(int(offs[g]), kg)]
                    for b in range(BC):
                        ap = u_t[:, ds(b * kg, kg)]
                        nc.vector._custom_dve(SCAN_OP, out=ap, in0=ap, in1=scn_g)
                # h_final = last scan element per (lane, b) -> bf16
                hsrc = uwin.rearrange("p (b t) -> p b t", b=BC)[:, :, kg - 1]
                nc.vector.tensor_copy(h_all[:, ds(g * BC, BC)], hsrc)

            # --- final projection: Y = h^T @ WHOT + bias ---
            # (emitted after all main-GEMM matmuls so no PE-FIFO stall; chunk 0
            # last, so the tail after its scan is a single matmul)
            psy = fpool.tile([BC, O], f32, tag="fy", name="psy")
            for i, g in enumerate(chunk_order):
                nc.tensor.matmul(
                    psy[:],
                    h_all[:, ds(g * BC, BC)],
                    whot_t[:, ds(g * O, O)],
                    start=(i == 0),
                    stop=(i == NCH - 1),
                )
            y_t = ypool.tile([BC, O], f32, tag="y", name="y_t")
            nc.vector.tensor_tensor(y_t[:], psy[:], bias_t[:], mybir.AluOpType.add)
            nc.sync.dma_start(Y, y_t[:])
    nc.compile()
    return nc


def _get_program(windows, offs_total):
    key = (
        windows,
        os.environ.get("DIAG_R"),
        os.environ.get("DIAG_LN"),
    )
    if key not in _CACHE:
        _CACHE[key] = _build(windows, offs_total)
    return _CACHE[key]


def _ensure_ntff_hook():
    """Provide antenv.axon_hooks (absent in this image) so trace=True works."""
    import sys
    import types

    if "antenv.axon_hooks" in sys.modules:
        return True
    try:
        import antenv

        mod = types.ModuleType("antenv.axon_hooks")
        mod._hook = None

        def set_axon_ntff_profile_hook(h):
            mod._hook = h

        def get_axon_ntff_profile_hook():
            return mod._hook

        mod.set_axon_ntff_profile_hook = set_axon_ntff_profile_hook
        mod.get_axon_ntff_profile_hook = get_axon_ntff_profile_hook
        sys.modules["antenv.axon_hooks"] = mod
        antenv.axon_hooks = mod

        from trn_agent_boot.trn_boot import _ntff_profile_via_ctypes

        hook = _ntff_profile_via_ctypes("/opt/axon/libaxon_pjrt.so")
        mod.set_axon_ntff_profile_hook(hook)
        return hook is not None
    except Exception:
        return False


def kernel(X, W_ih, hh, W_ho, b_ho):
    import ml_dtypes

    from concourse import bass_utils

    X = np.asarray(X, dtype=np.float32)
    W_ih = np.asarray(W_ih, dtype=np.float32)
    hh = np.asarray(hh, dtype=np.float32)
    W_ho = np.asarray(W_ho, dtype=np.float32)
    b_ho = np.asarray(b_ho, dtype=np.float32)

    plan = _make_plan(hh)
    perm = plan["perm"]
    nc = _get_program(plan["windows"], int(plan["offs"][-1]))

    bf = ml_dtypes.bfloat16
    # WIHT [128, NCH*NI*128]: line p = [g, ic, hsub], W_ih[h=g*128+hsub, i=ic*128+p]
    wiht = np.ascontiguousarray(
        W_ih[perm].T.reshape(NI, 128, NCH, 128).transpose(1, 2, 0, 3).reshape(128, -1)
    ).astype(bf)
    # WHOT [128, NCH*O]: line p = [g, o] with value W_ho[o, h=g*128+p]
    whot = np.ascontiguousarray(
        W_ho[:, perm].T.reshape(NCH, 128, O).transpose(1, 0, 2).reshape(128, NCH * O)
    ).astype(bf)
    bias = np.tile(b_ho[None, :], (BC, 1)).astype(np.float32)

    common = {
        "WIHT": wiht,
        "WHOT": whot,
        "BIAS": bias,
        "SCN": plan["SCN"].astype(bf),
    }
    in_maps = []
    for m in range(NCORES):
        im = dict(common)
        xm = X[:, m * BC : (m + 1) * BC, :]  # [S, BC, I]
        # device layout [NBLK, 128(i-in-chunk), (ic, b, tau)]
        xt = xm.transpose(2, 1, 0).reshape(NI, 128, BC, NBLK, TB)
        xt = np.ascontiguousarray(xt.transpose(3, 1, 0, 2, 4)).reshape(
            NBLK, 128, NI * BC * TB
        )
        im["X"] = xt.astype(bf)
        in_maps.append(im)

    trace = bool(int(os.environ.get("DIAG_TRACE", "0")))
    if trace:
        trace = _ensure_ntff_hook()
    res = None
    for attempt in range(3):
        try:
            res = bass_utils.run_bass_kernel_spmd(
                nc,
                in_maps,
                core_ids=list(range(NCORES)),
                trace=trace,
                tmpdir=os.environ.get("DIAG_TRACE_DIR") or None,
            )
            break
        except Exception:
            if attempt == 2:
                raise
            trace = False  # retry without profiling
    if res.exec_time_ns is not None:
        kernel.last_exec_time_ns = res.exec_time_ns
        kernel.last_mean_exec_time_ns = res.mean_exec_time_ns
    Yfull = np.concatenate([r["Y"] for r in res.results], axis=0)
    return Yfull


kernel.last_exec_time_ns = None
kernel.last_mean_exec_time_ns = None
